# revision 5
# baseline (speedup 1.0000x reference)
"""BRF (bursting resonate-and-fire) neuron update kernel for Trainium2.

Computes, elementwise over [B=4096, D=4096] fp32 tensors (per-neuron
vectors omegas/bs/threshold along D):

    omega  = |omegas|
    p      = (-1 + sqrt(1 - (DT*omega)^2)) / DT
    b      = p - |bs| - q
    u_     = u + b*u*DT - omega*v*DT + x*DT
    v_new  = v + omega*u*DT + b*v*DT
    z      = heaviside(|u_| - |threshold| - q)
    q_new  = q*0.9 + z

Sharding: batch rows split evenly across 8 NeuronCores (data parallel,
contiguous row slabs). Per-neuron [D] vectors fold host-side into
    C  = 1 + DT*(p - |bs|)    W = DT*omega    TH = |threshold|
so that, with A = C - DT*q:
    u_ = A*u - W*v + DT*x     v_ = A*v + W*u
    z  = (|u_| > TH + q)      q_ = 0.9*q + z

Precision scheme (validated: worst relerr ~3.4e-3 vs f32 reference,
dominated by 1-2 z bit flips out of ~88k spikes; gate is 2e-2):
  * x, v are shipped to the device as fp16 (their contribution to u_ is
    O(DT) / tolerance-checked); u, q stay fp32 (z threshold-critical).
  * The u_ chain is computed in fp32; z compares fp32 |u_| vs fp32 TH+q.
  * The v_ chain runs entirely in fp16 (DVE 2x mode) off fp16 copies of
    A and u made by the ACT engine.
  * Stores: u_/v_/q_/z fp16 (0/1 exact in fp16; the HW verifier
    rejects integer-out is_gt on Pool); host upconverts to fp32.

Engine balance per [128,2048] tile (cost-model ns; Pool ISA allows only
plain TensorTensor — no scalar_tensor_tensor, no PSUM, no integer out):
  GPS  A=C+(-DT*q) 4351 | thq=T+q 4351 | q_=qd16+z 4351  -> 13.1us
  DVE  p1=A*u, u1=p1-p2, uo=stt(x,DT,u1)->PSUM, z=is_gt->f16 (f32
       2388ea) | p2=W16*v, p3=A16*v, p4=W16*u16, vb (f16 2x 1455ea) -> 15.4us
  ACT  -DT*q, A16, u16, ub, au=|uo|, qd16=0.9*q (copies/scales) -> 12.7us
  DMA  3MB loads + 2MB stores                                  -> ~12us
HW-calibrated (repeat-slope, dispatch overhead cancelled): see test.py.
"""

import os

import numpy as np

DT = 1.0 / 24000.0
Q_DECAY = 0.9
B, D = 4096, 4096
N_CORES = 8
ROWS = B // N_CORES  # rows per core
P = 128  # SBUF partitions

LAST_EXEC_TIME_NS = None
LAST_RESULTS = None


def _legalize_bir_waits(raw: bytes) -> bytes:
    """Split multi-wait instructions into EventSemaphore + 1-wait instruction.

    The walrus build in this toolchain encodes at most ONE sync-wait per
    instruction; Tile's semaphore assignment emits several. Hoisting the
    extra waits onto standalone EventSemaphore instructions immediately
    before the instruction (same engine stream, in-order) is semantically
    identical.
    """
    import json

    d = json.loads(raw)
    for fn in d.get("functions", []):
        for bb in fn.get("blocks", []):
            out = []
            for ins in bb.get("instructions", []):
                si = ins.get("sync_info") or {}
                waits = si.get("on_wait") or []
                if len(waits) > 1:
                    for k, w in enumerate(waits[:-1]):
                        out.append(
                            {
                                "debug": ins.get("debug", 0),
                                "engine": ins["engine"],
                                "ins": [],
                                "name": f"{ins['name']}-w{k}",
                                "opcode": "EventSemaphore",
                                "outs": [],
                                "sync_info": {"on_update": [], "on_wait": [w]},
                            }
                        )
                    si["on_wait"] = [waits[-1]]
                out.append(ins)
            bb["instructions"] = out
    return json.dumps(d).encode()


def _install_wait_legalizer(nc):
    orig = nc.to_json_bytes

    def patched():
        return _legalize_bir_waits(orig())

    nc.to_json_bytes = patched
    return nc


def build_nc(rows=ROWS, d=D, free=2048, repeat=1, io_bufs=2, tmp_bufs=2):
    """Per-core Bass program (identical on all 8 cores). repeat>1 re-emits
    the main loop for slope-based timing only."""
    import concourse.bass as bass
    import concourse.mybir as mybir
    from concourse.tile import TileContext

    f32 = mybir.dt.float32
    f16 = mybir.dt.float16
    u8 = mybir.dt.uint8
    Alu = mybir.AluOpType
    Act = mybir.ActivationFunctionType

    nc = bass.Bass(trn_type="TRN2")

    x = nc.dram_tensor("x", [rows, d], f16, kind="ExternalInput")
    u = nc.dram_tensor("u", [rows, d], f32, kind="ExternalInput")
    v = nc.dram_tensor("v", [rows, d], f16, kind="ExternalInput")
    q = nc.dram_tensor("q", [rows, d], f32, kind="ExternalInput")
    cvec = nc.dram_tensor("cvec", [1, d], f32, kind="ExternalInput")
    w16vec = nc.dram_tensor("w16vec", [1, d], f16, kind="ExternalInput")
    tvec = nc.dram_tensor("tvec", [1, d], f32, kind="ExternalInput")

    z_o = nc.dram_tensor("z_o", [rows, d], f16, kind="ExternalOutput")
    u_o = nc.dram_tensor("u_o", [rows, d], f16, kind="ExternalOutput")
    v_o = nc.dram_tensor("v_o", [rows, d], f16, kind="ExternalOutput")
    q_o = nc.dram_tensor("q_o", [rows, d], f16, kind="ExternalOutput")

    n_pb = rows // P
    n_fc = d // free

    with TileContext(nc) as tc:
        with (
            tc.tile_pool(name="consts", bufs=1) as cp,
            tc.tile_pool(name="io", bufs=io_bufs) as iop,
            tc.tile_pool(name="tmp", bufs=tmp_bufs) as tp,
            tc.tile_pool(name="ps", bufs=2, space="PSUM") as pp,
        ):
            # Broadcast per-neuron vectors to all 128 partitions.
            Ct = cp.tile([P, d], f32, tag="C")
            W16t = cp.tile([P, d], f16, tag="W16")
            Tt = cp.tile([P, d], f32, tag="T")
            for tile, handle in ((Ct, cvec), (W16t, w16vec), (Tt, tvec)):
                src = handle[:]
                bc = bass.AP(tensor=src.tensor, offset=src.offset, ap=[[0, P], [1, d]])
                nc.gpsimd.dma_start(out=tile[:], in_=bc)

            for pb in range(n_pb * repeat):
                r0 = (pb % n_pb) * P
                for fc in range(n_fc):
                    c0 = fc * free
                    rs = slice(r0, r0 + P)
                    cs = slice(c0, c0 + free)

                    xt = iop.tile([P, free], f16, tag="x")
                    ut = iop.tile([P, free], f32, tag="u")
                    vt = iop.tile([P, free], f16, tag="v")
                    qt = iop.tile([P, free], f32, tag="q")
                    nc.sync.dma_start(out=xt[:], in_=x[rs, cs])
                    nc.sync.dma_start(out=ut[:], in_=u[rs, cs])
                    nc.sync.dma_start(out=vt[:], in_=v[rs, cs])
                    nc.sync.dma_start(out=qt[:], in_=q[rs, cs])

                    Cc = Ct[:, cs]
                    W16c = W16t[:, cs]
                    Tc = Tt[:, cs]

                    # A = C + (-DT*q): scale on ACT, add on GPS
                    qdneg = tp.tile([P, free], f32, tag="t8a")
                    nc.scalar.activation(qdneg[:], qt[:], Act.Copy, bias=0.0, scale=-DT)
                    At = tp.tile([P, free], f32, tag="t8A")
                    nc.gpsimd.tensor_tensor(At[:], Cc, qdneg[:], Alu.add)
                    # ACT: f16 copies for the 2x v-chain
                    A16 = tp.tile([P, free], f16, tag="t4A")
                    nc.scalar.activation(A16[:], At[:], Act.Copy)
                    u16 = tp.tile([P, free], f16, tag="t4u")
                    nc.scalar.activation(u16[:], ut[:], Act.Copy)
                    # DVE u-chain (p2 f16 2x, rest f32)
                    p2 = tp.tile([P, free], f16, tag="t4p2")
                    nc.vector.tensor_mul(p2[:], W16c, vt[:])
                    p1 = tp.tile([P, free], f32, tag="t8a")
                    nc.vector.tensor_mul(p1[:], At[:], ut[:])
                    u1 = tp.tile([P, free], f32, tag="t8c")
                    nc.vector.tensor_sub(u1[:], p1[:], p2[:])
                    uo = pp.tile([P, free], f32, tag="uo")
                    nc.vector.scalar_tensor_tensor(
                        uo[:], xt[:], DT, u1[:], Alu.mult, Alu.add
                    )
                    # DVE v-chain, all-f16 (2x mode)
                    p3 = tp.tile([P, free], f16, tag="t4a")
                    nc.vector.tensor_mul(p3[:], A16[:], vt[:])
                    p4 = tp.tile([P, free], f16, tag="t4b")
                    nc.vector.tensor_mul(p4[:], W16c, u16[:])
                    nc.vector.tensor_tensor(p3[:], p3[:], p4[:], Alu.add)
                    vb = p3
                    # ACT: u_ f16 store copy + |u_| (after p4 so tags recycle)
                    ub = tp.tile([P, free], f16, tag="t4u")
                    nc.scalar.activation(ub[:], uo[:], Act.Copy)
                    au = tp.tile([P, free], f32, tag="t8a")
                    nc.scalar.activation(au[:], uo[:], Act.Abs)
                    # GPS: thq, z (f16 out)
                    thq = tp.tile([P, free], f32, tag="t8c")
                    nc.gpsimd.tensor_tensor(thq[:], Tc, qt[:], Alu.add)
                    zz = tp.tile([P, free], f16, tag="t4A")
                    nc.vector.tensor_tensor(zz[:], au[:], thq[:], Alu.is_gt)
                    # q_new = 0.9q + z: scale on ACT, all-f16 add on DVE
                    qd16 = tp.tile([P, free], f16, tag="t4p2")
                    nc.scalar.activation(qd16[:], qt[:], Act.Copy, bias=0.0, scale=Q_DECAY)
                    nc.gpsimd.tensor_tensor(p4[:], qd16[:], zz[:], Alu.add)
                    qb = p4

                    nc.scalar.dma_start(out=u_o[rs, cs], in_=ub[:])
                    nc.scalar.dma_start(out=v_o[rs, cs], in_=vb[:])
                    nc.scalar.dma_start(out=z_o[rs, cs], in_=zz[:])
                    nc.scalar.dma_start(out=q_o[rs, cs], in_=qb[:])

    return _install_wait_legalizer(nc)


def host_consts(omegas, bs, threshold):
    """Fold the per-neuron vectors into C/W/TH (fp32, matching jax order)."""
    f = np.float32
    om = np.abs(omegas.astype(np.float32))
    w = (f(DT) * om).astype(np.float32)  # DT*omega
    p = ((f(-1.0) + np.sqrt((f(1.0) - w * w).astype(np.float32))) / f(DT)).astype(
        np.float32
    )
    c1 = (p - np.abs(bs.astype(np.float32))).astype(np.float32)
    c = (f(1.0) + (f(DT) * c1).astype(np.float32)).astype(np.float32)
    th = np.abs(threshold.astype(np.float32))
    d = om.shape[0]
    return c.reshape(1, d), w.reshape(1, d), th.reshape(1, d)


_NC_CACHE = {}


def kernel(x, u, v, q, omegas, bs, threshold):
    global LAST_EXEC_TIME_NS, LAST_RESULTS
    from concourse import bass_utils

    cvec, wvec, tvec = host_consts(omegas, bs, threshold)
    w16vec = wvec.astype(np.float16)

    if "nc" not in _NC_CACHE:
        _NC_CACHE["nc"] = build_nc()
    nc = _NC_CACHE["nc"]

    x16 = np.ascontiguousarray(x, dtype=np.float32).astype(np.float16)
    v16 = np.ascontiguousarray(v, dtype=np.float32).astype(np.float16)
    u = np.ascontiguousarray(u, dtype=np.float32)
    q = np.ascontiguousarray(q, dtype=np.float32)

    in_maps = []
    for k in range(N_CORES):
        sl = slice(k * ROWS, (k + 1) * ROWS)
        in_maps.append(
            {
                "x": x16[sl],
                "u": u[sl],
                "v": v16[sl],
                "q": q[sl],
                "cvec": cvec,
                "w16vec": w16vec,
                "tvec": tvec,
            }
        )

    trace = bool(int(os.environ.get("BRF_TRACE", "0")))
    res = bass_utils.run_bass_kernel_spmd(
        nc, in_maps, core_ids=list(range(N_CORES)), trace=trace
    )
    LAST_EXEC_TIME_NS = res.exec_time_ns
    LAST_RESULTS = res

    zf = np.concatenate(
        [res.results[k]["z_o"] for k in range(N_CORES)], axis=0
    ).astype(np.float32)
    uf = np.concatenate(
        [res.results[k]["u_o"] for k in range(N_CORES)], axis=0
    ).astype(np.float32)
    vf = np.concatenate(
        [res.results[k]["v_o"] for k in range(N_CORES)], axis=0
    ).astype(np.float32)
    qf = np.concatenate(
        [res.results[k]["q_o"] for k in range(N_CORES)], axis=0
    ).astype(np.float32)
    return (zf, uf, vf, qf)


# revision 7
# speedup vs baseline: 1.0780x; 1.0780x over previous
"""BRF (bursting resonate-and-fire) neuron update kernel for Trainium2.

Computes, elementwise over [B=4096, D=4096] fp32 tensors (per-neuron
vectors omegas/bs/threshold along D):

    omega  = |omegas|
    p      = (-1 + sqrt(1 - (DT*omega)^2)) / DT
    b      = p - |bs| - q
    u_     = u + b*u*DT - omega*v*DT + x*DT
    v_new  = v + omega*u*DT + b*v*DT
    z      = heaviside(|u_| - |threshold| - q)
    q_new  = q*0.9 + z

Sharding: batch rows split evenly across 8 NeuronCores (data parallel,
contiguous row slabs). Per-neuron [D] vectors fold host-side into
    C  = 1 + DT*(p - |bs|)    W = DT*omega    TH = |threshold|
so that, with A = C - DT*q:
    u_ = A*u - W*v + DT*x     v_ = A*v + W*u
    z  = (|u_| > TH + q)      q_ = 0.9*q + z

Precision scheme (validated: worst relerr ~3.4e-3 vs f32 reference,
dominated by 1-2 z bit flips out of ~88k spikes; gate is 2e-2):
  * x, v are shipped to the device as fp16 (their contribution to u_ is
    O(DT) / tolerance-checked); u, q stay fp32 (z threshold-critical).
  * The u_ chain is computed in fp32; z compares fp32 |u_| vs fp32 TH+q.
  * The v_ chain runs entirely in fp16 (DVE 2x mode) off fp16 copies of
    A and u made by the ACT engine.
  * Stores: u_/v_/q_/z fp16 (0/1 exact in fp16; the HW verifier
    rejects integer-out is_gt on Pool); host upconverts to fp32.

Engine balance per [128,2048] tile (cost-model ns; Pool ISA allows only
plain TensorTensor — no scalar_tensor_tensor, no PSUM, no integer out):
  GPS  A=C+(-DT*q) 4351 | thq=T+q 4351 (stage1 only, never gates) -> 8.7us
  DVE  p1=A*u, u1=p1-p2, uo=stt(x,DT,u1)->PSUM, z=is_gt->f16 (f32
       2388ea) | p2,p3,p4,vb,q_=qd16+z (f16 2x 1455ea)          -> 16.8us
  ACT  -DT*q, A16, u16, ub, au=|uo|, qd16=0.9*q (copies/scales) -> 12.7us
  DMA  3MB loads + 2MB stores                                  -> ~12us
The per-tile op DAG is software-pipelined: each iteration emits
loads(k+1), stage2(k), stage1(k+1), so early-rank ops of tile k+1 sit
ahead of tile k's late tail in every engine's (in-order) queue — without
this the per-tile critical path (~22us) serializes tile to tile.
HW-calibrated (repeat-slope, dispatch overhead cancelled): see test.py.
"""

import os

import numpy as np

DT = 1.0 / 24000.0
Q_DECAY = 0.9
B, D = 4096, 4096
N_CORES = 8
ROWS = B // N_CORES  # rows per core
P = 128  # SBUF partitions

LAST_EXEC_TIME_NS = None
LAST_RESULTS = None


def _legalize_bir_waits(raw: bytes) -> bytes:
    """Split multi-wait instructions into EventSemaphore + 1-wait instruction.

    The walrus build in this toolchain encodes at most ONE sync-wait per
    instruction; Tile's semaphore assignment emits several. Hoisting the
    extra waits onto standalone EventSemaphore instructions immediately
    before the instruction (same engine stream, in-order) is semantically
    identical.
    """
    import json

    d = json.loads(raw)
    for fn in d.get("functions", []):
        for bb in fn.get("blocks", []):
            out = []
            for ins in bb.get("instructions", []):
                si = ins.get("sync_info") or {}
                waits = si.get("on_wait") or []
                if len(waits) > 1:
                    for k, w in enumerate(waits[:-1]):
                        out.append(
                            {
                                "debug": ins.get("debug", 0),
                                "engine": ins["engine"],
                                "ins": [],
                                "name": f"{ins['name']}-w{k}",
                                "opcode": "EventSemaphore",
                                "outs": [],
                                "sync_info": {"on_update": [], "on_wait": [w]},
                            }
                        )
                    si["on_wait"] = [waits[-1]]
                out.append(ins)
            bb["instructions"] = out
    return json.dumps(d).encode()


def _install_wait_legalizer(nc):
    orig = nc.to_json_bytes

    def patched():
        return _legalize_bir_waits(orig())

    nc.to_json_bytes = patched
    return nc


def build_nc(rows=ROWS, d=D, free=2048, repeat=1, io_bufs=2, tmp_bufs=2):
    """Per-core Bass program (identical on all 8 cores). repeat>1 re-emits
    the main loop for slope-based timing only."""
    import concourse.bass as bass
    import concourse.mybir as mybir
    from concourse.tile import TileContext

    f32 = mybir.dt.float32
    f16 = mybir.dt.float16
    u8 = mybir.dt.uint8
    Alu = mybir.AluOpType
    Act = mybir.ActivationFunctionType

    nc = bass.Bass(trn_type="TRN2")

    x = nc.dram_tensor("x", [rows, d], f16, kind="ExternalInput")
    u = nc.dram_tensor("u", [rows, d], f32, kind="ExternalInput")
    v = nc.dram_tensor("v", [rows, d], f16, kind="ExternalInput")
    q = nc.dram_tensor("q", [rows, d], f32, kind="ExternalInput")
    cvec = nc.dram_tensor("cvec", [1, d], f32, kind="ExternalInput")
    w16vec = nc.dram_tensor("w16vec", [1, d], f16, kind="ExternalInput")
    tvec = nc.dram_tensor("tvec", [1, d], f32, kind="ExternalInput")

    z_o = nc.dram_tensor("z_o", [rows, d], f16, kind="ExternalOutput")
    u_o = nc.dram_tensor("u_o", [rows, d], f16, kind="ExternalOutput")
    v_o = nc.dram_tensor("v_o", [rows, d], f16, kind="ExternalOutput")
    q_o = nc.dram_tensor("q_o", [rows, d], f16, kind="ExternalOutput")

    n_pb = rows // P
    n_fc = d // free

    with TileContext(nc) as tc:
        with (
            tc.tile_pool(name="consts", bufs=1) as cp,
            tc.tile_pool(name="io", bufs=io_bufs) as iop,
            tc.tile_pool(name="tmp", bufs=tmp_bufs) as tp,
            tc.tile_pool(name="ps", bufs=2, space="PSUM") as pp,
        ):
            # Broadcast per-neuron vectors to all 128 partitions.
            Ct = cp.tile([P, d], f32, tag="C")
            W16t = cp.tile([P, d], f16, tag="W16")
            Tt = cp.tile([P, d], f32, tag="T")
            for tile, handle in ((Ct, cvec), (W16t, w16vec), (Tt, tvec)):
                src = handle[:]
                bc = bass.AP(tensor=src.tensor, offset=src.offset, ap=[[0, P], [1, d]])
                nc.gpsimd.dma_start(out=tile[:], in_=bc)

            def slices(k):
                pb = (k // n_fc) % n_pb
                fc = k % n_fc
                return slice(pb * P, pb * P + P), slice(fc * free, fc * free + free)

            n_tiles = n_pb * n_fc * repeat
            st = [None] * n_tiles

            def loads(k):
                rs, cs = slices(k)
                xt = iop.tile([P, free], f16, tag="x")
                ut = iop.tile([P, free], f32, tag="u")
                vt = iop.tile([P, free], f16, tag="v")
                qt = iop.tile([P, free], f32, tag="q")
                nc.sync.dma_start(out=xt[:], in_=x[rs, cs])
                nc.sync.dma_start(out=ut[:], in_=u[rs, cs])
                nc.sync.dma_start(out=vt[:], in_=v[rs, cs])
                nc.sync.dma_start(out=qt[:], in_=q[rs, cs])
                st[k] = dict(xt=xt, ut=ut, vt=vt, qt=qt)

            def stage1(k):
                # early rank: everything that needs only the loads
                s = st[k]
                rs, cs = slices(k)
                Cc, W16c, Tc = Ct[:, cs], W16t[:, cs], Tt[:, cs]
                qdneg = tp.tile([P, free], f32, tag="t8a")
                nc.scalar.activation(qdneg[:], s["qt"][:], Act.Copy, bias=0.0, scale=-DT)
                At = tp.tile([P, free], f32, tag="t8A")
                nc.gpsimd.tensor_tensor(At[:], Cc, qdneg[:], Alu.add)
                A16 = tp.tile([P, free], f16, tag="t4A")
                nc.scalar.activation(A16[:], At[:], Act.Copy)
                u16 = tp.tile([P, free], f16, tag="t4u")
                nc.scalar.activation(u16[:], s["ut"][:], Act.Copy)
                p2 = tp.tile([P, free], f16, tag="t4p2")
                nc.vector.tensor_mul(p2[:], W16c, s["vt"][:])
                p1 = tp.tile([P, free], f32, tag="t8a")
                nc.vector.tensor_mul(p1[:], At[:], s["ut"][:])
                thq = tp.tile([P, free], f32, tag="t8c")
                nc.gpsimd.tensor_tensor(thq[:], Tc, s["qt"][:], Alu.add)
                qd16 = tp.tile([P, free], f16, tag="t4p2")
                nc.scalar.activation(qd16[:], s["qt"][:], Act.Copy, bias=0.0, scale=Q_DECAY)
                s.update(A16=A16, u16=u16, p2=p2, p1=p1, thq=thq, qd16=qd16)

            def stage2(k):
                # late rank: the dependent tail + stores
                s = st[k]
                rs, cs = slices(k)
                W16c = W16t[:, cs]
                u1 = tp.tile([P, free], f32, tag="t8c")
                nc.vector.tensor_sub(u1[:], s["p1"][:], s["p2"][:])
                uo = pp.tile([P, free], f32, tag="uo")
                nc.vector.scalar_tensor_tensor(
                    uo[:], s["xt"][:], DT, u1[:], Alu.mult, Alu.add
                )
                p3 = tp.tile([P, free], f16, tag="t4a")
                nc.vector.tensor_mul(p3[:], s["A16"][:], s["vt"][:])
                p4 = tp.tile([P, free], f16, tag="t4b")
                nc.vector.tensor_mul(p4[:], W16c, s["u16"][:])
                nc.vector.tensor_tensor(p3[:], p3[:], p4[:], Alu.add)
                vb = p3
                ub = tp.tile([P, free], f16, tag="t4u")
                nc.scalar.activation(ub[:], uo[:], Act.Copy)
                au = tp.tile([P, free], f32, tag="t8a")
                nc.scalar.activation(au[:], uo[:], Act.Abs)
                zz = tp.tile([P, free], f16, tag="t4A")
                nc.vector.tensor_tensor(zz[:], au[:], s["thq"][:], Alu.is_gt)
                nc.vector.tensor_tensor(p4[:], s["qd16"][:], zz[:], Alu.add)
                qb = p4
                nc.scalar.dma_start(out=u_o[rs, cs], in_=ub[:])
                nc.scalar.dma_start(out=v_o[rs, cs], in_=vb[:])
                nc.scalar.dma_start(out=z_o[rs, cs], in_=zz[:])
                nc.scalar.dma_start(out=q_o[rs, cs], in_=qb[:])
                st[k] = None

            loads(0)
            stage1(0)
            for k in range(n_tiles):
                if k + 1 < n_tiles:
                    loads(k + 1)
                stage2(k)
                if k + 1 < n_tiles:
                    stage1(k + 1)

    return _install_wait_legalizer(nc)


def host_consts(omegas, bs, threshold):
    """Fold the per-neuron vectors into C/W/TH (fp32, matching jax order)."""
    f = np.float32
    om = np.abs(omegas.astype(np.float32))
    w = (f(DT) * om).astype(np.float32)  # DT*omega
    p = ((f(-1.0) + np.sqrt((f(1.0) - w * w).astype(np.float32))) / f(DT)).astype(
        np.float32
    )
    c1 = (p - np.abs(bs.astype(np.float32))).astype(np.float32)
    c = (f(1.0) + (f(DT) * c1).astype(np.float32)).astype(np.float32)
    th = np.abs(threshold.astype(np.float32))
    d = om.shape[0]
    return c.reshape(1, d), w.reshape(1, d), th.reshape(1, d)


_NC_CACHE = {}


def kernel(x, u, v, q, omegas, bs, threshold):
    global LAST_EXEC_TIME_NS, LAST_RESULTS
    from concourse import bass_utils

    cvec, wvec, tvec = host_consts(omegas, bs, threshold)
    w16vec = wvec.astype(np.float16)

    if "nc" not in _NC_CACHE:
        _NC_CACHE["nc"] = build_nc()
    nc = _NC_CACHE["nc"]

    x16 = np.ascontiguousarray(x, dtype=np.float32).astype(np.float16)
    v16 = np.ascontiguousarray(v, dtype=np.float32).astype(np.float16)
    u = np.ascontiguousarray(u, dtype=np.float32)
    q = np.ascontiguousarray(q, dtype=np.float32)

    in_maps = []
    for k in range(N_CORES):
        sl = slice(k * ROWS, (k + 1) * ROWS)
        in_maps.append(
            {
                "x": x16[sl],
                "u": u[sl],
                "v": v16[sl],
                "q": q[sl],
                "cvec": cvec,
                "w16vec": w16vec,
                "tvec": tvec,
            }
        )

    trace = bool(int(os.environ.get("BRF_TRACE", "0")))
    res = bass_utils.run_bass_kernel_spmd(
        nc, in_maps, core_ids=list(range(N_CORES)), trace=trace
    )
    LAST_EXEC_TIME_NS = res.exec_time_ns
    LAST_RESULTS = res

    zf = np.concatenate(
        [res.results[k]["z_o"] for k in range(N_CORES)], axis=0
    ).astype(np.float32)
    uf = np.concatenate(
        [res.results[k]["u_o"] for k in range(N_CORES)], axis=0
    ).astype(np.float32)
    vf = np.concatenate(
        [res.results[k]["v_o"] for k in range(N_CORES)], axis=0
    ).astype(np.float32)
    qf = np.concatenate(
        [res.results[k]["q_o"] for k in range(N_CORES)], axis=0
    ).astype(np.float32)
    return (zf, uf, vf, qf)
